# revision 50
# baseline (speedup 1.0000x reference)
"""Multi-head attention (RoPE) Trainium2 kernel, 8-way sharded.

Sharding: core c handles batch b = c//4 and 4 heads h0 = 4*(c%4).

Per-core program (v4 — pipelined around the ScalarE exp floor):
  Heads are processed as two pairs (0,1) and (2,3). Post-rope q/k live in
  pair tiles [128, 2048] with the even head in partitions 0-63 and the odd
  head in 64-127, so the two heads' score matmuls (K=64) run CONCURRENTLY
  as PE row-tiles (0,0)/(64,0) — 2x score throughput.

  PSUM (8 banks): sp0/sp1 score tiles [128,1024] f32 (2+2 banks, one per
  head, ping-ponged by the exp consumer), pv0/pv1 [65,512] accumulators
  (1+1), and two 1-bank projection chains (A/B) through which qkv pieces,
  v pieces and out-proj pieces flow as fillers under the exp-bound
  attention pipeline.

  Inputs are host-folded so every tensor loads with one (or four)
  128-row fully-contiguous DMA (issue cost is descriptor-bound).

  attention loop, software-pipelined (scores one step ahead; PE FIFO
  order per step is [sp0' | pv0 | sp1' | pv1 | fillers]):
    S^T[kt,q] row-tiled pair -> exp(0.125 S) f16 (2 ACT calls, FD=1024)
    -> PV accumulate [V|1]^T @ P^T into pv (row 64 = softmax denominator)
  norm per (pair, qq): recip(denominator) -> gpsimd broadcast -> DVE mul
  out_proj per token block: y = attn_out^T @ w_out slices, evac, DMA.

  host: y[b] = sum of the 4 per-core partials (fp32).
"""

import numpy as np

B = 2
N = 2048
C = 1024
HD = 64
HC = 4  # heads per core
N_CORES = 8
ROPE_BASE = 10000.0

_PROGRAM = None


def _rope_tables():
    inv_freq = 1.0 / (ROPE_BASE ** (np.arange(0, HD, 2, dtype=np.float32) / HD))
    t = np.arange(N, dtype=np.float32)
    freqs = np.einsum("i,j->ij", t, inv_freq).astype(np.float32)  # [N, 32]
    emb = np.concatenate([freqs, freqs], axis=-1)  # [N, 64]
    cos = np.cos(emb).astype(np.float32)
    sin = np.sin(emb).astype(np.float32)
    cosT = np.ascontiguousarray(np.tile(cos.T, (2, 1)))  # [128, 2048]
    sinT = sin.T.copy()  # [64, 2048]
    sinT_signed = np.concatenate([-sinT[:32], sinT[32:]], axis=0)
    sinT2 = np.ascontiguousarray(np.tile(sinT_signed, (2, 1)))  # [128, 2048]
    return cosT, sinT2


def _fold(a, chunks):
    """[chunks*128, F] -> [128, chunks*F] partition-contiguous layout."""
    ch, rem = a.shape[0] // 128, a.shape[1]
    assert ch == chunks
    return np.ascontiguousarray(
        a.reshape(chunks, 128, rem).transpose(1, 0, 2).reshape(128, chunks * rem)
    )


def _build_program():
    import concourse.mybir as mybir
    import concourse.tile as tile
    from concourse import bacc

    f32 = mybir.dt.float32
    f16 = mybir.dt.float16
    f8 = mybir.dt.float8e4
    DR = mybir.MatmulPerfMode.DoubleRow
    MUL = mybir.AluOpType.mult
    ADD = mybir.AluOpType.add
    EXP = mybir.ActivationFunctionType.Exp

    nc = bacc.Bacc("TRN2", target_bir_lowering=False, debug=False, num_devices=N_CORES)

    xT_d = nc.dram_tensor("xTq", [128, 4 * 8 * 512], f16, kind="ExternalInput").ap()
    wqk_d = nc.dram_tensor("wqkF", [128, 8 * 512], f16, kind="ExternalInput").ap()
    wv_d = nc.dram_tensor("wvF", [128, 8 * 256], f16, kind="ExternalInput").ap()
    wo_d = nc.dram_tensor("woF", [128, 2 * C], f16, kind="ExternalInput").ap()
    cos_d = nc.dram_tensor("cosT", [128, N], f32, kind="ExternalInput").ap()
    sin_d = nc.dram_tensor("sinT", [128, N], f32, kind="ExternalInput").ap()
    y_d = nc.dram_tensor("y", [N, C], f16, kind="ExternalOutput").ap()

    with tile.TileContext(nc) as tc:
        with (
            tc.tile_pool(name="persist", bufs=1) as persist,
            tc.tile_pool(name="work", bufs=2) as work,
            tc.tile_pool(name="psum", bufs=1, space="PSUM") as psp,
        ):
            # ---------------- persistent SBUF ----------------
            xT = persist.tile([128, 4, 8, 512], f16, tag="xT", name="xT")
            wqk = persist.tile([128, 8, 512], f16, tag="wqk", name="wqk")
            wv = persist.tile([128, 8, 256], f16, tag="wv", name="wv")
            wo = persist.tile([128, 2, C], f16, tag="wo", name="wo")
            cosT = persist.tile([128, N], f32, tag="cosT", name="cosT")
            sinT = persist.tile([128, N], f32, tag="sinT", name="sinT")
            # q-pair0, q-pair1, k-pair0, k-pair1  (matches wqkT col blocks)
            qk = [
                persist.tile([128, N], f16, tag=f"qk{t}", name=f"qk{t}")
                for t in range(4)
            ]
            vv = persist.tile([128, 16, HC, HD + 1], f16, tag="vv", name="vv")
            ao = [
                persist.tile([128, N], f16, tag=f"ao{p}", name=f"ao{p}")
                for p in range(2)
            ]

            # ---------------- helpers ----------------
            def dma_in():
                # one fully-contiguous 128-row DMA per tensor (quarters for
                # xT), ordered earliest-needed first: the first matmuls need
                # wqk+xT[q0], the first rope needs cos/sin cols 0-511
                nc.sync.dma_start(cosT[:, 0:512], cos_d[:, 0:512])
                nc.sync.dma_start(sinT[:, 0:512], sin_d[:, 0:512])
                nc.sync.dma_start(wqk[:], wqk_d[:, :])
                nc.sync.dma_start(xT[:, 0], xT_d[:, 0:4096])
                nc.sync.dma_start(wv[:], wv_d[:, :])
                nc.sync.dma_start(xT[:, 1], xT_d[:, 4096:8192])
                nc.sync.dma_start(xT[:, 2], xT_d[:, 8192:12288])
                nc.sync.dma_start(xT[:, 3], xT_d[:, 12288:16384])
                nc.sync.dma_start(cosT[:, 512:], cos_d[:, 512:])
                nc.sync.dma_start(sinT[:, 512:], sin_d[:, 512:])
                nc.sync.dma_start(wo[:], wo_d[:, :])

            wsc = persist.tile([128, 512], f16, tag="wsc", name="wsc")

            def pe_warmup(tag, nmm):
                # junk matmuls on a zeroed scratch to hold the PE HAM clock
                # gate at 8/8 across idle stretches
                wps = psp.tile([128, 512], f32, tag=tag, name="wps")
                for r in range(nmm):
                    nc.tensor.matmul(
                        wps[:], wsc[:, 0:128], wsc[:],
                        start=(r == 0), stop=(r == nmm - 1),
                    )

            def act_table_preload():
                scratch = work.tile([128, 16], f32, tag="dmy", name="dmy")
                nc.vector.memset(scratch[:], 0.0)
                dmye = work.tile([128, 16], f16, tag="dmye", name="dmye")
                nc.scalar.activation(dmye[:], scratch[:], EXP)

            def qk_piece(t, pc, chain, part=None):
                """project + rope one 512-token piece of qk tile t.
                part=None: whole piece; part=(state, 0/1): half for smooth
                filler interleave (4 matmuls per half, rope with part 1)."""
                sl = slice(pc * 512, (pc + 1) * 512)
                if part is None or part[1] == 0:
                    bp = psp.tile([128, 512], f32, tag=chain, name=f"bp{t}_{pc}")
                    if part is not None:
                        part[0]["bp"] = bp
                else:
                    bp = part[0]["bp"]
                cts = range(8) if part is None else (
                    range(4) if part[1] == 0 else range(4, 8)
                )
                for ct in cts:
                    nc.tensor.matmul(
                        bp[:],
                        wqk[:, ct, t * 128 : (t + 1) * 128],
                        xT[:, pc, ct, :],
                        start=(ct == 0),
                        stop=(ct == 7),
                    )
                if part is not None and part[1] == 0:
                    return
                t_sb = work.tile([128, 512], f32, tag="ropet", name="rt")
                u_sb = work.tile([128, 512], f32, tag="ropeu", name="ru")
                nc.vector.tensor_tensor(t_sb[:], bp[:], cosT[:, sl], MUL)
                for o_lo, i_lo in [(0, 32), (32, 0), (64, 96), (96, 64)]:
                    nc.vector.tensor_tensor(
                        u_sb[o_lo : o_lo + 32, :],
                        bp[i_lo : i_lo + 32, :],
                        sinT[o_lo : o_lo + 32, sl],
                        MUL,
                    )
                nc.vector.tensor_tensor(qk[t][:, sl], t_sb[:], u_sb[:], ADD)

            def fill_qk2(pair_qq_i_a, pair_qq_i_b, t, pc, chain):
                """schedule one qk piece as two 4-matmul halves."""
                st = {}
                add_fill(*pair_qq_i_a, lambda: qk_piece(t, pc, chain, (st, 0)))
                add_fill(*pair_qq_i_b, lambda: qk_piece(t, pc, chain, (st, 1)))

            def v_piece(tt, chain, on_scalar=False, part=None):
                """V' tile for one 128-token block (token-major)."""
                if part is None or part[1] == 0:
                    vp = psp.tile([128, 256], f32, tag=chain, name=f"vp{tt}")
                    if part is not None:
                        part[0]["vp"] = vp
                else:
                    vp = part[0]["vp"]
                tsl = slice((tt % 4) * 128, (tt % 4) * 128 + 128)
                cts = range(8) if part is None else (
                    range(4) if part[1] == 0 else range(4, 8)
                )
                for ct in cts:
                    nc.tensor.matmul(
                        vp[:],
                        xT[:, tt // 4, ct, tsl],
                        wv[:, ct, :],
                        start=(ct == 0),
                        stop=(ct == 7),
                    )
                if part is not None and part[1] == 0:
                    return
                dst = vv[:, tt, :, 0:HD]
                srcap = vp[:].rearrange("p (h d) -> p h d", h=HC)
                if on_scalar:
                    nc.scalar.copy(dst, srcap)
                else:
                    nc.vector.tensor_copy(dst, srcap)

            def y_piece(tt, oc, chain, on_scalar=False):
                osl = slice(oc * 512, (oc + 1) * 512)
                yps = psp.tile([128, 512], f32, tag=chain, name=f"yps{tt}_{oc}")
                for p in range(2):
                    nc.tensor.matmul(
                        yps[:],
                        ao[p][:, tt * 128 : (tt + 1) * 128],
                        wo[:, p, osl],
                        start=(p == 0),
                        stop=(p == 1),
                    )
                ysb = work.tile([128, 512], f16, tag="ysb", bufs=3, name="ysb")
                if on_scalar:
                    nc.scalar.copy(ysb[:], yps[:])
                else:
                    nc.vector.tensor_copy(ysb[:], yps[:])
                nc.sync.dma_start(y_d[tt * 128 : (tt + 1) * 128, osl], ysb[:])

            # filler schedule: maps (pair, qq, i) -> list of thunks.
            # deadlines carry >= 1 qq-window of margin (chain latency is
            # ~5.8us per qk piece: 1.7 matmul + 4.1 rope DVE)
            fillers = {}

            def add_fill(pair, qq, i, fn):
                fillers.setdefault((pair, qq, i), []).append(fn)

            # pair0 window: v14/v15, rest of qk0, all of qk1, as half-pieces
            def fill_v2(a, b, tt, chain):
                st = {}
                add_fill(*a, lambda: v_piece(tt, chain, part=(st, 0)))
                add_fill(*b, lambda: v_piece(tt, chain, part=(st, 1)))

            fill_v2((0, 0, 0), (0, 0, 1), 14, "pA")
            fill_qk2((0, 0, 0), (0, 0, 1), 2, 2, "pB")
            fill_v2((0, 0, 2), (0, 0, 3), 15, "pA")
            fill_qk2((0, 0, 2), (0, 0, 3), 2, 3, "pB")
            fill_qk2((0, 0, 4), (0, 0, 5), 0, 1, "pA")
            fill_qk2((0, 0, 6), (0, 0, 7), 3, 0, "pB")
            fill_qk2((0, 1, 0), (0, 1, 0), 0, 2, "pA")
            fill_qk2((0, 1, 2), (0, 1, 3), 3, 1, "pB")
            fill_qk2((0, 1, 4), (0, 1, 5), 1, 0, "pA")
            fill_qk2((0, 1, 6), (0, 1, 7), 3, 2, "pB")
            fill_qk2((0, 2, 0), (0, 2, 0), 0, 3, "pA")
            fill_qk2((0, 2, 2), (0, 2, 3), 3, 3, "pB")
            fill_qk2((0, 2, 4), (0, 2, 5), 1, 1, "pA")
            fill_qk2((0, 3, 0), (0, 3, 0), 1, 2, "pA")
            fill_qk2((0, 3, 4), (0, 3, 5), 1, 3, "pA")
            # pair1 window: out-proj pieces for the previous qq's token blocks
            for qq in range(1, 4):
                for j in range(4):
                    tt = (qq - 1) * 4 + j
                    add_fill(1, qq, j + 1, (lambda t: lambda: y_piece(t, 0, "pA"))(tt))
                    add_fill(1, qq, j + 1, (lambda t: lambda: y_piece(t, 1, "pB"))(tt))

            def norm(pair, qq, pv0, pv1):
                # ao[pair][j*64:(j+1)*64, qsl] = pv_j[0:64] / pv_j[64]
                qsl = slice(qq * 512, (qq + 1) * 512)
                for j, pv in ((0, pv0), (1, pv1)):
                    rr = work.tile([1, 512], f32, tag="rr", name="rr")
                    nc.vector.tensor_copy(rr[:], pv[HD : HD + 1, :])
                    ra = work.tile([1, 512], f32, tag="ra", name="ra")
                    nc.vector.reciprocal_approx_fast(ra[:], rr[:])
                    nb = work.tile([64, 512], f32, tag="nb", name="nb")
                    nc.gpsimd.partition_broadcast(nb[:], ra[:])
                    nc.vector.tensor_tensor(
                        ao[pair][j * 64 : (j + 1) * 64, qsl],
                        pv[0:HD, :],
                        nb[:],
                        MUL,
                    )

            def emit_S_head(pair, qq, i, which):
                """score matmuls for one head of (pair, qq, ktpair).
                which=0: even head -> sp0 (PE rows 0-63); which=1: odd -> sp1."""
                qt = qk[pair]
                kt_ = qk[2 + pair]
                qsl = slice(qq * 512, (qq + 1) * 512)
                lo, hi = (0, 64) if which == 0 else (64, 128)
                sp = psp.tile(
                    [128, 1024], f32, tag=f"sp{which}", name=f"sp{which}_{pair}_{qq}_{i}"
                )
                for half, kk in ((0, 2 * i), (1, 2 * i + 1)):
                    ksl = slice(kk * 128, (kk + 1) * 128)
                    ssl = slice(half * 512, (half + 1) * 512)
                    nc.tensor.matmul(
                        sp[:, ssl], kt_[lo:hi, ksl], qt[lo:hi, qsl],
                        start=True, stop=True,
                    )
                return sp

            def attention(pairs):
                # software-pipelined: score matmuls one step ahead, PE FIFO
                # order [sp0' | pv0 | sp1' | pv1 | fillers] so the ACT (exp)
                # stream never stalls behind PV/filler matmuls
                steps = [
                    (pair, qq, i) for pair in pairs for qq in range(4) for i in range(8)
                ]
                sp0 = emit_S_head(*steps[0], 0)
                sp1 = emit_S_head(*steps[0], 1)
                pvs = {}
                for n, (pair, qq, i) in enumerate(steps):
                    if i == 0:
                        pvs[pair, qq] = (
                            psp.tile([HD + 1, 512], f32, tag="pv0", name=f"pv0_{pair}_{qq}"),
                            psp.tile([HD + 1, 512], f32, tag="pv1", name=f"pv1_{pair}_{qq}"),
                        )
                    pv0, pv1 = pvs[pair, qq]
                    es0 = work.tile([128, 1024], f16, tag="es0", bufs=3, name="es0")
                    es1 = work.tile([128, 1024], f16, tag="es1", bufs=3, name="es1")
                    nc.scalar.activation(es0[:], sp0[:], EXP, scale=float(HD**-0.5))
                    nc.scalar.activation(es1[:], sp1[:], EXP, scale=float(HD**-0.5))
                    nxt = steps[n + 1] if n + 1 < len(steps) else None

                    def emit_pv(pvt, vvh, es):
                        for half, kk in ((0, 2 * i), (1, 2 * i + 1)):
                            nc.tensor.matmul(
                                pvt[:], vv[:, kk, vvh, :],
                                es[:, half * 512 : (half + 1) * 512],
                                start=(i == 0 and half == 0),
                                stop=(i == 7 and half == 1),
                            )

                    if i == 0:
                        # boundary: first PV matmuls wait for the previous
                        # quarter's norm (pv slot WAR); emit both S heads and
                        # the fillers first so the exp stream and the next
                        # iter's scores aren't head-blocked behind them
                        if nxt:
                            sp0 = emit_S_head(*nxt, 0)
                            sp1 = emit_S_head(*nxt, 1)
                        for fn in fillers.get((pair, qq, i), ()):
                            fn()
                        emit_pv(pv0, 2 * pair, es0)
                        emit_pv(pv1, 2 * pair + 1, es1)
                    else:
                        if nxt:
                            sp0 = emit_S_head(*nxt, 0)
                        emit_pv(pv0, 2 * pair, es0)
                        if nxt:
                            sp1 = emit_S_head(*nxt, 1)
                        emit_pv(pv1, 2 * pair + 1, es1)
                        if i == 7:
                            norm(pair, qq, pv0, pv1)
                        for fn in fillers.get((pair, qq, i), ()):
                            fn()

            # ---------------- emission ----------------
            act_table_preload()
            nc.vector.memset(wsc[:], 0.0)
            pe_warmup("sp0", 8)
            dma_in()
            nc.vector.memset(vv[:, :, :, HD : HD + 1], 1.0)
            # boot: k-pair0 cols 0-1023, q-pair0 cols 0-511, v blocks 0-9.
            # Six independent psum tags (pv0/pv1 are free until attention) in
            # dependency-ready emission order; v-copies go to the idle
            # ScalarE so the DVE queue holds only the 18 boot rope ops.
            qk_piece(2, 0, "pB")
            qk_piece(0, 0, "pA")
            v_piece(0, "sp0", True)
            v_piece(1, "sp1", True)
            qk_piece(2, 1, "pv0")
            v_piece(2, "pv1", True)
            v_piece(3, "sp0", True)
            v_piece(4, "sp1", True)
            v_piece(5, "pv1", True)
            v_piece(6, "pB", True)
            v_piece(7, "pA", True)
            v_piece(8, "pv0", True)
            v_piece(9, "pv1", True)
            v_piece(10, "sp0", True)
            v_piece(11, "sp1", True)
            v_piece(12, "pA", True)
            v_piece(13, "pB", True)

            attention((0, 1))

            pe_warmup("pB", 6)
            # tail: out-proj for the last quarter's token blocks, 4 chains,
            # evac copies split across VectorE and the now-idle ScalarE
            for j, chain, on_sc in (
                (0, "pA", False),
                (1, "pB", False),
                (2, "sp0", True),
                (3, "sp1", True),
            ):
                tt = 12 + j
                y_piece(tt, 0, chain, on_sc)
                y_piece(tt, 1, chain, on_sc)

    nc.compile()
    return nc


def _get_program():
    global _PROGRAM
    if _PROGRAM is None:
        _PROGRAM = _build_program()
    return _PROGRAM


def _make_in_maps(x, w_qkv, w_out):
    x = np.asarray(x, dtype=np.float32)
    w_qkv = np.asarray(w_qkv, dtype=np.float32)
    w_out = np.asarray(w_out, dtype=np.float32)
    cosT, sinT = _rope_tables()
    in_maps = []
    for c in range(N_CORES):
        b = c // 4
        h0 = HC * (c % 4)
        rows = np.arange(h0 * HD, (h0 + HC) * HD)
        wq = w_qkv[rows]  # [256, 1024]
        wk = w_qkv[C + rows]
        wv = w_qkv[2 * C + rows]
        xT_c = np.ascontiguousarray(x[b].T).astype(np.float16)  # [1024, 2048]
        # xT quarters folded: [128, (pc, ct, f)]
        xTq = np.ascontiguousarray(
            xT_c.reshape(8, 128, 4, 512).transpose(1, 2, 0, 3).reshape(128, -1)
        )
        wqkT = np.concatenate([wq, wk], 0).T.astype(np.float16)  # [1024, 512]
        woT = w_out[:, rows].T.astype(np.float16)  # [256, 1024]
        in_maps.append(
            {
                "xTq": xTq,
                "wqkF": _fold(wqkT, 8),
                "wvF": _fold(wv.T.astype(np.float16), 8),
                "woF": _fold(woT, 2),
                "cosT": cosT,
                "sinT": sinT,
            }
        )
    return in_maps


def run(inputs, trace=False, trace_cores=None):
    from concourse.bass_utils import run_bass_kernel_spmd

    nc = _get_program()
    in_maps = _make_in_maps(inputs["x"], inputs["w_qkv"], inputs["w_out"])
    res = run_bass_kernel_spmd(
        nc,
        in_maps,
        core_ids=list(range(N_CORES)),
        trace=trace,
        trace_cores=trace_cores,
    )
    y = np.zeros((B, N, C), dtype=np.float32)
    for c in range(N_CORES):
        y[c // 4] += res.results[c]["y"].astype(np.float32)
    return y, res


def kernel(**inputs) -> np.ndarray:
    y, _ = run(inputs, trace=False)
    return y


# revision 51
# speedup vs baseline: 1.0127x; 1.0127x over previous
"""Multi-head attention (RoPE) Trainium2 kernel, 8-way sharded.

Sharding: core c handles batch b = c//4 and 4 heads h0 = 4*(c%4).

Per-core program (v4 — pipelined around the ScalarE exp floor):
  Heads are processed as two pairs (0,1) and (2,3). Post-rope q/k live in
  pair tiles [128, 2048] with the even head in partitions 0-63 and the odd
  head in 64-127, so the two heads' score matmuls (K=64) run CONCURRENTLY
  as PE row-tiles (0,0)/(64,0) — 2x score throughput.

  PSUM (8 banks): sp0/sp1 score tiles [128,1024] f32 (2+2 banks, one per
  head, ping-ponged by the exp consumer), pv0/pv1 [65,512] accumulators
  (1+1), and two 1-bank projection chains (A/B) through which qkv pieces,
  v pieces and out-proj pieces flow as fillers under the exp-bound
  attention pipeline.

  Inputs are host-folded so every tensor loads with one (or four)
  128-row fully-contiguous DMA (issue cost is descriptor-bound).

  attention loop, software-pipelined (scores one step ahead; PE FIFO
  order per step is [sp0' | pv0 | sp1' | pv1 | fillers]):
    S^T[kt,q] row-tiled pair -> exp(0.125 S) f16 (2 ACT calls, FD=1024)
    -> PV accumulate [V|1]^T @ P^T into pv (row 64 = softmax denominator)
  norm per (pair, qq): recip(denominator) -> gpsimd broadcast -> DVE mul
  out_proj per token block: y = attn_out^T @ w_out slices, evac, DMA.

  host: y[b] = sum of the 4 per-core partials (fp32).
"""

import numpy as np

B = 2
N = 2048
C = 1024
HD = 64
HC = 4  # heads per core
N_CORES = 8
ROPE_BASE = 10000.0

_PROGRAM = None


def _rope_tables():
    inv_freq = 1.0 / (ROPE_BASE ** (np.arange(0, HD, 2, dtype=np.float32) / HD))
    t = np.arange(N, dtype=np.float32)
    freqs = np.einsum("i,j->ij", t, inv_freq).astype(np.float32)  # [N, 32]
    emb = np.concatenate([freqs, freqs], axis=-1)  # [N, 64]
    cos = np.cos(emb).astype(np.float32)
    sin = np.sin(emb).astype(np.float32)
    cosT = np.ascontiguousarray(np.tile(cos.T, (2, 1)))  # [128, 2048]
    sinT = sin.T.copy()  # [64, 2048]
    sinT_signed = np.concatenate([-sinT[:32], sinT[32:]], axis=0)
    sinT2 = np.ascontiguousarray(np.tile(sinT_signed, (2, 1)))  # [128, 2048]
    return cosT, sinT2


def _fold(a, chunks):
    """[chunks*128, F] -> [128, chunks*F] partition-contiguous layout."""
    ch, rem = a.shape[0] // 128, a.shape[1]
    assert ch == chunks
    return np.ascontiguousarray(
        a.reshape(chunks, 128, rem).transpose(1, 0, 2).reshape(128, chunks * rem)
    )


def _build_program():
    import concourse.mybir as mybir
    import concourse.tile as tile
    from concourse import bacc

    f32 = mybir.dt.float32
    f16 = mybir.dt.float16
    f8 = mybir.dt.float8e4
    DR = mybir.MatmulPerfMode.DoubleRow
    MUL = mybir.AluOpType.mult
    ADD = mybir.AluOpType.add
    EXP = mybir.ActivationFunctionType.Exp

    nc = bacc.Bacc("TRN2", target_bir_lowering=False, debug=False, num_devices=N_CORES)

    xT_d = nc.dram_tensor("xTq", [128, 4 * 8 * 512], f16, kind="ExternalInput").ap()
    wqk_d = nc.dram_tensor("wqkF", [128, 8 * 512], f16, kind="ExternalInput").ap()
    wv_d = nc.dram_tensor("wvF", [128, 8 * 256], f16, kind="ExternalInput").ap()
    wo_d = nc.dram_tensor("woF", [128, 2 * C], f16, kind="ExternalInput").ap()
    cos_d = nc.dram_tensor("cosT", [128, N], f32, kind="ExternalInput").ap()
    sin_d = nc.dram_tensor("sinT", [128, N], f32, kind="ExternalInput").ap()
    y_d = nc.dram_tensor("y", [N, C], f16, kind="ExternalOutput").ap()

    with tile.TileContext(nc) as tc:
        with (
            tc.tile_pool(name="persist", bufs=1) as persist,
            tc.tile_pool(name="work", bufs=2) as work,
            tc.tile_pool(name="psum", bufs=1, space="PSUM") as psp,
        ):
            # ---------------- persistent SBUF ----------------
            xT = persist.tile([128, 4, 8, 512], f16, tag="xT", name="xT")
            wqk = persist.tile([128, 8, 512], f16, tag="wqk", name="wqk")
            wv = persist.tile([128, 8, 256], f16, tag="wv", name="wv")
            wo = persist.tile([128, 2, C], f16, tag="wo", name="wo")
            cosT = persist.tile([128, N], f32, tag="cosT", name="cosT")
            sinT = persist.tile([128, N], f32, tag="sinT", name="sinT")
            # q-pair0, q-pair1, k-pair0, k-pair1  (matches wqkT col blocks)
            qk = [
                persist.tile([128, N], f16, tag=f"qk{t}", name=f"qk{t}")
                for t in range(4)
            ]
            vv = persist.tile([128, 16, HC, HD + 1], f16, tag="vv", name="vv")
            ao = [
                persist.tile([128, N], f16, tag=f"ao{p}", name=f"ao{p}")
                for p in range(2)
            ]

            # ---------------- helpers ----------------
            def dma_in():
                # one fully-contiguous 128-row DMA per tensor (quarters for
                # xT), ordered earliest-needed first: the first matmuls need
                # wqk+xT[q0], the first rope needs cos/sin cols 0-511
                nc.sync.dma_start(wqk[:], wqk_d[:, :])
                nc.sync.dma_start(xT[:, 0], xT_d[:, 0:4096])
                nc.sync.dma_start(cosT[:, 0:512], cos_d[:, 0:512])
                nc.sync.dma_start(sinT[:, 0:512], sin_d[:, 0:512])
                nc.sync.dma_start(wv[:], wv_d[:, :])
                nc.sync.dma_start(xT[:, 1], xT_d[:, 4096:8192])
                nc.sync.dma_start(xT[:, 2], xT_d[:, 8192:12288])
                nc.sync.dma_start(xT[:, 3], xT_d[:, 12288:16384])
                nc.sync.dma_start(cosT[:, 512:], cos_d[:, 512:])
                nc.sync.dma_start(sinT[:, 512:], sin_d[:, 512:])
                nc.sync.dma_start(wo[:], wo_d[:, :])

            def pe_warmup():
                # ~8 junk matmuls on a zeroed scratch so the PE HAM clock
                # gate is at 8/8 by the time the real pipeline starts
                wsc = work.tile([128, 512], f16, tag="wsc", name="wsc")
                nc.vector.memset(wsc[:], 0.0)
                wps = psp.tile([128, 512], f32, tag="sp0", name="wps")
                for r in range(8):
                    nc.tensor.matmul(
                        wps[:], wsc[:, 0:128], wsc[:], start=(r == 0), stop=(r == 7)
                    )

            def act_table_preload():
                scratch = work.tile([128, 16], f32, tag="dmy", name="dmy")
                nc.vector.memset(scratch[:], 0.0)
                dmye = work.tile([128, 16], f16, tag="dmye", name="dmye")
                nc.scalar.activation(dmye[:], scratch[:], EXP)

            def qk_piece(t, pc, chain, part=None):
                """project + rope one 512-token piece of qk tile t.
                part=None: whole piece; part=(state, 0/1): half for smooth
                filler interleave (4 matmuls per half, rope with part 1)."""
                sl = slice(pc * 512, (pc + 1) * 512)
                if part is None or part[1] == 0:
                    bp = psp.tile([128, 512], f32, tag=chain, name=f"bp{t}_{pc}")
                    if part is not None:
                        part[0]["bp"] = bp
                else:
                    bp = part[0]["bp"]
                cts = range(8) if part is None else (
                    range(4) if part[1] == 0 else range(4, 8)
                )
                for ct in cts:
                    nc.tensor.matmul(
                        bp[:],
                        wqk[:, ct, t * 128 : (t + 1) * 128],
                        xT[:, pc, ct, :],
                        start=(ct == 0),
                        stop=(ct == 7),
                    )
                if part is not None and part[1] == 0:
                    return
                t_sb = work.tile([128, 512], f32, tag="ropet", name="rt")
                u_sb = work.tile([128, 512], f32, tag="ropeu", name="ru")
                nc.vector.tensor_tensor(t_sb[:], bp[:], cosT[:, sl], MUL)
                for o_lo, i_lo in [(0, 32), (32, 0), (64, 96), (96, 64)]:
                    nc.vector.tensor_tensor(
                        u_sb[o_lo : o_lo + 32, :],
                        bp[i_lo : i_lo + 32, :],
                        sinT[o_lo : o_lo + 32, sl],
                        MUL,
                    )
                nc.vector.tensor_tensor(qk[t][:, sl], t_sb[:], u_sb[:], ADD)

            def fill_qk2(pair_qq_i_a, pair_qq_i_b, t, pc, chain):
                """schedule one qk piece as two 4-matmul halves."""
                st = {}
                add_fill(*pair_qq_i_a, lambda: qk_piece(t, pc, chain, (st, 0)))
                add_fill(*pair_qq_i_b, lambda: qk_piece(t, pc, chain, (st, 1)))

            def v_piece(tt, chain, on_scalar=False, part=None):
                """V' tile for one 128-token block (token-major)."""
                if part is None or part[1] == 0:
                    vp = psp.tile([128, 256], f32, tag=chain, name=f"vp{tt}")
                    if part is not None:
                        part[0]["vp"] = vp
                else:
                    vp = part[0]["vp"]
                tsl = slice((tt % 4) * 128, (tt % 4) * 128 + 128)
                cts = range(8) if part is None else (
                    range(4) if part[1] == 0 else range(4, 8)
                )
                for ct in cts:
                    nc.tensor.matmul(
                        vp[:],
                        xT[:, tt // 4, ct, tsl],
                        wv[:, ct, :],
                        start=(ct == 0),
                        stop=(ct == 7),
                    )
                if part is not None and part[1] == 0:
                    return
                dst = vv[:, tt, :, 0:HD]
                srcap = vp[:].rearrange("p (h d) -> p h d", h=HC)
                if on_scalar:
                    nc.scalar.copy(dst, srcap)
                else:
                    nc.vector.tensor_copy(dst, srcap)

            def y_piece(tt, oc, chain, on_scalar=False):
                osl = slice(oc * 512, (oc + 1) * 512)
                yps = psp.tile([128, 512], f32, tag=chain, name=f"yps{tt}_{oc}")
                for p in range(2):
                    nc.tensor.matmul(
                        yps[:],
                        ao[p][:, tt * 128 : (tt + 1) * 128],
                        wo[:, p, osl],
                        start=(p == 0),
                        stop=(p == 1),
                    )
                ysb = work.tile([128, 512], f16, tag="ysb", bufs=3, name="ysb")
                if on_scalar:
                    nc.scalar.copy(ysb[:], yps[:])
                else:
                    nc.vector.tensor_copy(ysb[:], yps[:])
                nc.sync.dma_start(y_d[tt * 128 : (tt + 1) * 128, osl], ysb[:])

            # filler schedule: maps (pair, qq, i) -> list of thunks.
            # deadlines carry >= 1 qq-window of margin (chain latency is
            # ~5.8us per qk piece: 1.7 matmul + 4.1 rope DVE)
            fillers = {}

            def add_fill(pair, qq, i, fn):
                fillers.setdefault((pair, qq, i), []).append(fn)

            # pair0 window: v14/v15, rest of qk0, all of qk1, as half-pieces
            def fill_v2(a, b, tt, chain):
                st = {}
                add_fill(*a, lambda: v_piece(tt, chain, part=(st, 0)))
                add_fill(*b, lambda: v_piece(tt, chain, part=(st, 1)))

            fill_v2((0, 0, 0), (0, 0, 1), 14, "pA")
            fill_qk2((0, 0, 0), (0, 0, 1), 2, 2, "pB")
            fill_v2((0, 0, 2), (0, 0, 3), 15, "pA")
            fill_qk2((0, 0, 2), (0, 0, 3), 2, 3, "pB")
            fill_qk2((0, 0, 4), (0, 0, 5), 0, 1, "pA")
            fill_qk2((0, 0, 6), (0, 0, 7), 3, 0, "pB")
            fill_qk2((0, 1, 0), (0, 1, 1), 0, 2, "pA")
            fill_qk2((0, 1, 2), (0, 1, 3), 3, 1, "pB")
            fill_qk2((0, 1, 4), (0, 1, 5), 1, 0, "pA")
            fill_qk2((0, 1, 6), (0, 1, 7), 3, 2, "pB")
            fill_qk2((0, 2, 0), (0, 2, 1), 0, 3, "pA")
            fill_qk2((0, 2, 2), (0, 2, 3), 3, 3, "pB")
            fill_qk2((0, 2, 4), (0, 2, 5), 1, 1, "pA")
            fill_qk2((0, 3, 0), (0, 3, 1), 1, 2, "pA")
            fill_qk2((0, 3, 4), (0, 3, 5), 1, 3, "pA")
            # pair1 window: out-proj pieces for the previous qq's token blocks
            for qq in range(1, 4):
                for j in range(4):
                    tt = (qq - 1) * 4 + j
                    add_fill(1, qq, j + 1, (lambda t: lambda: y_piece(t, 0, "pA"))(tt))
                    add_fill(1, qq, j + 1, (lambda t: lambda: y_piece(t, 1, "pB"))(tt))

            def norm(pair, qq, pv0, pv1):
                # ao[pair][j*64:(j+1)*64, qsl] = pv_j[0:64] / pv_j[64]
                qsl = slice(qq * 512, (qq + 1) * 512)
                for j, pv in ((0, pv0), (1, pv1)):
                    rr = work.tile([1, 512], f32, tag="rr", name="rr")
                    nc.vector.tensor_copy(rr[:], pv[HD : HD + 1, :])
                    ra = work.tile([1, 512], f32, tag="ra", name="ra")
                    nc.vector.reciprocal_approx_fast(ra[:], rr[:])
                    nb = work.tile([64, 512], f32, tag="nb", name="nb")
                    nc.gpsimd.partition_broadcast(nb[:], ra[:])
                    nc.vector.tensor_tensor(
                        ao[pair][j * 64 : (j + 1) * 64, qsl],
                        pv[0:HD, :],
                        nb[:],
                        MUL,
                    )

            def emit_S_head(pair, qq, i, which):
                """score matmuls for one head of (pair, qq, ktpair).
                which=0: even head -> sp0 (PE rows 0-63); which=1: odd -> sp1."""
                qt = qk[pair]
                kt_ = qk[2 + pair]
                qsl = slice(qq * 512, (qq + 1) * 512)
                lo, hi = (0, 64) if which == 0 else (64, 128)
                sp = psp.tile(
                    [128, 1024], f32, tag=f"sp{which}", name=f"sp{which}_{pair}_{qq}_{i}"
                )
                for half, kk in ((0, 2 * i), (1, 2 * i + 1)):
                    ksl = slice(kk * 128, (kk + 1) * 128)
                    ssl = slice(half * 512, (half + 1) * 512)
                    nc.tensor.matmul(
                        sp[:, ssl], kt_[lo:hi, ksl], qt[lo:hi, qsl],
                        start=True, stop=True,
                    )
                return sp

            def attention(pairs):
                # software-pipelined: score matmuls one step ahead, PE FIFO
                # order [sp0' | pv0 | sp1' | pv1 | fillers] so the ACT (exp)
                # stream never stalls behind PV/filler matmuls
                steps = [
                    (pair, qq, i) for pair in pairs for qq in range(4) for i in range(8)
                ]
                sp0 = emit_S_head(*steps[0], 0)
                sp1 = emit_S_head(*steps[0], 1)
                pvs = {}
                for n, (pair, qq, i) in enumerate(steps):
                    if i == 0:
                        pvs[pair, qq] = (
                            psp.tile([HD + 1, 512], f32, tag="pv0", name=f"pv0_{pair}_{qq}"),
                            psp.tile([HD + 1, 512], f32, tag="pv1", name=f"pv1_{pair}_{qq}"),
                        )
                    pv0, pv1 = pvs[pair, qq]
                    es0 = work.tile([128, 1024], f16, tag="es0", bufs=3, name="es0")
                    es1 = work.tile([128, 1024], f16, tag="es1", bufs=3, name="es1")
                    nc.scalar.activation(es0[:], sp0[:], EXP, scale=float(HD**-0.5))
                    nc.scalar.activation(es1[:], sp1[:], EXP, scale=float(HD**-0.5))
                    nxt = steps[n + 1] if n + 1 < len(steps) else None

                    def emit_pv(pvt, vvh, es):
                        for half, kk in ((0, 2 * i), (1, 2 * i + 1)):
                            nc.tensor.matmul(
                                pvt[:], vv[:, kk, vvh, :],
                                es[:, half * 512 : (half + 1) * 512],
                                start=(i == 0 and half == 0),
                                stop=(i == 7 and half == 1),
                            )

                    if i == 0:
                        # boundary: first PV matmuls wait for the previous
                        # quarter's norm (pv slot WAR); emit both S heads and
                        # the fillers first so the exp stream and the next
                        # iter's scores aren't head-blocked behind them
                        if nxt:
                            sp0 = emit_S_head(*nxt, 0)
                            sp1 = emit_S_head(*nxt, 1)
                        for fn in fillers.get((pair, qq, i), ()):
                            fn()
                        emit_pv(pv0, 2 * pair, es0)
                        emit_pv(pv1, 2 * pair + 1, es1)
                    else:
                        if nxt:
                            sp0 = emit_S_head(*nxt, 0)
                        emit_pv(pv0, 2 * pair, es0)
                        if nxt:
                            sp1 = emit_S_head(*nxt, 1)
                        emit_pv(pv1, 2 * pair + 1, es1)
                        if i == 7:
                            norm(pair, qq, pv0, pv1)
                        for fn in fillers.get((pair, qq, i), ()):
                            fn()

            # ---------------- emission ----------------
            act_table_preload()
            pe_warmup()
            dma_in()
            nc.vector.memset(vv[:, :, :, HD : HD + 1], 1.0)
            # boot: k-pair0 cols 0-1023, q-pair0 cols 0-511, v blocks 0-9.
            # Six independent psum tags (pv0/pv1 are free until attention) in
            # dependency-ready emission order; v-copies go to the idle
            # ScalarE so the DVE queue holds only the 18 boot rope ops.
            qk_piece(2, 0, "pB")
            qk_piece(0, 0, "pA")
            v_piece(0, "sp0", True)
            v_piece(1, "sp1", True)
            qk_piece(2, 1, "pv0")
            v_piece(2, "pv1", True)
            v_piece(3, "sp0", True)
            v_piece(4, "sp1", True)
            v_piece(5, "pv1", True)
            v_piece(6, "pB", True)
            v_piece(7, "pA", True)
            v_piece(8, "pv0", True)
            v_piece(9, "pv1", True)
            v_piece(10, "sp0", True)
            v_piece(11, "sp1", True)
            v_piece(12, "pA", True)
            v_piece(13, "pB", True)

            attention((0, 1))

            # tail: out-proj for the last quarter's token blocks, 4 chains,
            # evac copies split across VectorE and the now-idle ScalarE
            for j, chain, on_sc in (
                (0, "pA", False),
                (1, "pB", False),
                (2, "sp0", True),
                (3, "sp1", True),
            ):
                tt = 12 + j
                y_piece(tt, 0, chain, on_sc)
                y_piece(tt, 1, chain, on_sc)

    nc.compile()
    return nc


def _get_program():
    global _PROGRAM
    if _PROGRAM is None:
        _PROGRAM = _build_program()
    return _PROGRAM


def _make_in_maps(x, w_qkv, w_out):
    x = np.asarray(x, dtype=np.float32)
    w_qkv = np.asarray(w_qkv, dtype=np.float32)
    w_out = np.asarray(w_out, dtype=np.float32)
    cosT, sinT = _rope_tables()
    in_maps = []
    for c in range(N_CORES):
        b = c // 4
        h0 = HC * (c % 4)
        rows = np.arange(h0 * HD, (h0 + HC) * HD)
        wq = w_qkv[rows]  # [256, 1024]
        wk = w_qkv[C + rows]
        wv = w_qkv[2 * C + rows]
        xT_c = np.ascontiguousarray(x[b].T).astype(np.float16)  # [1024, 2048]
        # xT quarters folded: [128, (pc, ct, f)]
        xTq = np.ascontiguousarray(
            xT_c.reshape(8, 128, 4, 512).transpose(1, 2, 0, 3).reshape(128, -1)
        )
        wqkT = np.concatenate([wq, wk], 0).T.astype(np.float16)  # [1024, 512]
        woT = w_out[:, rows].T.astype(np.float16)  # [256, 1024]
        in_maps.append(
            {
                "xTq": xTq,
                "wqkF": _fold(wqkT, 8),
                "wvF": _fold(wv.T.astype(np.float16), 8),
                "woF": _fold(woT, 2),
                "cosT": cosT,
                "sinT": sinT,
            }
        )
    return in_maps


def run(inputs, trace=False, trace_cores=None):
    from concourse.bass_utils import run_bass_kernel_spmd

    nc = _get_program()
    in_maps = _make_in_maps(inputs["x"], inputs["w_qkv"], inputs["w_out"])
    res = run_bass_kernel_spmd(
        nc,
        in_maps,
        core_ids=list(range(N_CORES)),
        trace=trace,
        trace_cores=trace_cores,
    )
    y = np.zeros((B, N, C), dtype=np.float32)
    for c in range(N_CORES):
        y[c // 4] += res.results[c]["y"].astype(np.float32)
    return y, res


def kernel(**inputs) -> np.ndarray:
    y, _ = run(inputs, trace=False)
    return y
